# revision 23
# baseline (speedup 1.0000x reference)
"""Trainium2 Bass kernel for the differentiable isotropic-Gaussian renderer.

Math: the reference computes, per batch b,
    w[n, pix] = opac_n * exp(-0.5 * ||c_pix - proj_n||^2 / scales_n^2)
    out[c]    = (w.T @ colors) / (w.sum(0) + EPS)
Each gaussian is isotropic and the pixel grid is separable
(pix = (x, y), x in 0..W-1, y in 0..H-1), so the weight factorizes:
    w[n, (y,x)] = opac_n * Ey[n, y] * Ex[n, x]
    Ex[n, x] = exp(-((x - mx_n) * sqrt(.5)/s_n)^2),  Ey likewise.
The render collapses to 4 matmuls per image,
    S_j[y, x] = sum_n (q_j[n] * Ey[n, y]) * Ex[n, x],
with q_0 = opac (denominator), q_{1..3} = opac * color_c, then
out[c] = S_{c+1} / (S_0 + EPS).  This replaces the N x H*W dense weight
matrix (67M exps/batch) with N*(H+W) exps (0.5M/batch).

Sharding: 8 cores = 2 batches x 4 y-quarters (64 rows each).  Every core
gets the full gaussian set (replicated; tiny) plus a per-core projection
matrix and y-grid, computes its [3, 64, 256] slice entirely locally (no
collectives), and the host reassembles the [2, 3, 256, 256] output.

Perf notes (from the instruction-cost timeline sim):
 - render matmuls use float32r (full fp32 precision, 1 cyc/row when the
   moving dim >= 256, vs 4 cyc/row for plain fp32)
 - inputs are host-packed into two contiguous tensors so startup is a
   couple of small DMAs instead of seven strided ones
 - pixel-grid rows are DMA'd as single rows and replicated on-chip
 - the y-side (lhsT path: Ey -> q*Ey on GPSIMD) is emitted first so the
   PE can start while the x-side exps still run on ACT
"""

import numpy as np

import concourse.bacc as bacc
import bass_rust
import concourse.bass as bass
import concourse.tile as tile
from concourse import mybir
from concourse.bass_utils import run_bass_kernel_spmd

H, W = 256, 256
FX, FY = 300.0, 300.0
CX, CY = 128.0, 128.0
N = 1024
B = 2
EPS = 1e-8
NCORES = 8
YQ = H // 4          # y-rows per core
NCHUNK = N // 128    # gaussian partition chunks

TRACE = False
LAST_RESULTS = None
_CACHED_NC = None


def build_kernel(nc, sb, ps):
    f32 = mybir.dt.float32
    f32r = mybir.dt.float32r
    AT = mybir.AluOpType
    AF = mybir.ActivationFunctionType

    posT = nc.dram_tensor("post", [4, N], f32, kind="ExternalInput")
    gprops = nc.dram_tensor("gprops", [128, NCHUNK, 5], f32, kind="ExternalInput")
    # aux row: pm (12 floats) + xgrid (W) + ygrid (YQ)
    aux = nc.dram_tensor("aux", [1, 12 + W + YQ], f32, kind="ExternalInput")
    out = nc.dram_tensor("out", [3, YQ, W], f32, kind="ExternalOutput")

    # ---------------- input loads ----------------
    posTs = sb.tile([4, N], f32, tag="posTs")
    nc.sync.dma_start(out=posTs[:, :], in_=posT[:, :])
    pmt = sb.tile([4, 3], f32, tag="pmt")
    nc.sync.dma_start(
        out=pmt[:, :],
        in_=bass.AP(tensor=aux, offset=0, ap=[[3, 4], [1, 3]]),
    )
    gp = sb.tile([128, NCHUNK, 5], f32, tag="gp")
    nc.scalar.dma_start(out=gp[:, :, :], in_=gprops[:, :, :])
    grow = sb.tile([1, W + YQ], f32, tag="grow")
    nc.scalar.dma_start(
        out=grow[:, :],
        in_=bass.AP(tensor=aux, offset=12, ap=[[0, 1], [1, W + YQ]]),
    )

    # replicate grid rows across partitions (GPSIMD partition broadcast)
    yg = sb.tile([128, YQ], f32, tag="yg")
    nc.gpsimd.partition_broadcast(yg[:, :], grow[:, W : W + YQ])
    xg = sb.tile([128, W], f32, tag="xg")
    nc.gpsimd.partition_broadcast(xg[:, :], grow[:, 0:W])

    # ---------------- projection (PE) ----------------
    uvz_ps = ps.tile([128, NCHUNK * 3], f32, tag="uvz_ps")
    for c in range(NCHUNK):
        nc.tensor.matmul(
            uvz_ps[:, c * 3 : (c + 1) * 3],
            lhsT=posTs[:, c * 128 : (c + 1) * 128],
            rhs=pmt[:, :],
            start=True,
            stop=True,
        )
    uvz = uvz_ps.rearrange("p (c k) -> p c k", k=3)

    # keep the PE p-state warm between the projection and render matmuls
    # (otherwise the renders start at the throttled clock); results unused
    warm_ps = ps.tile([128, 112], f32, tag="warm_ps")
    for i in range(22):
        nc.tensor.matmul(
            warm_ps[:, :],
            lhsT=posTs[:, 0:128],
            rhs=posTs[:, 0:112],
            start=True,
            stop=True,
        )

    # ---------------- per-gaussian prep (DVE) ----------------
    rz = sb.tile([128, NCHUNK], f32, tag="rz")
    nc.vector.reciprocal(rz[:, :], uvz[:, :, 2])
    my = sb.tile([128, NCHUNK], f32, tag="my")
    nc.vector.tensor_mul(my[:, :], uvz[:, :, 1], rz[:, :])
    # sp = 1 / scale -> exponent = -0.5 * ((x - mx) * sp)^2 (0.5 folded
    # into the Exp scale)
    sp = sb.tile([128, NCHUNK], f32, tag="sp")
    nc.vector.reciprocal(sp[:, :], gp[:, :, 0])

    # ---------------- separable factors ----------------
    # y-side first: it feeds the matmul lhsT path (exp -> q*Ey on GPSIMD)
    ty = sb.tile([128, NCHUNK, YQ], f32, tag="ty")
    for c in range(NCHUNK):
        nc.vector.tensor_scalar(
            ty[:, c, :], yg[:, :], my[:, c : c + 1], sp[:, c : c + 1],
            op0=AT.subtract, op1=AT.mult,
        )
    ey = sb.tile([128, NCHUNK, YQ], f32, tag="ey")
    expy_is = []
    for h in range(2):
        hc = NCHUNK // 2
        yflat = ty[:, h * hc : (h + 1) * hc, :].rearrange("p c y -> p (c y)")
        eyflat = ey[:, h * hc : (h + 1) * hc, :].rearrange("p c y -> p (c y)")
        nc.scalar.activation(eyflat, yflat, AF.Square)
        expy_is.append(nc.scalar.activation(eyflat, eyflat, AF.Exp, scale=-0.5))
    expy_i = expy_is[-1]

    # x-side: fused affine on DVE, square on DVE (keeps ACT = exps only),
    # exp on ACT in two halves so the PE can start on the first half
    mx = sb.tile([128, NCHUNK], f32, tag="mx")
    nc.vector.tensor_mul(mx[:, :], uvz[:, :, 0], rz[:, :])
    tx = sb.tile([128, NCHUNK, W], f32, tag="tx")
    ex = sb.tile([128, NCHUNK, W], f32r, tag="ex")
    for c0, c1 in ((0, 2), (2, 4), (4, 6), (6, 8)):
        for c in range(c0, c1):
            nc.vector.tensor_scalar(
                tx[:, c, :], xg[:, :], mx[:, c : c + 1], sp[:, c : c + 1],
                op0=AT.subtract, op1=AT.mult,
            )
        txh = tx[:, c0:c1, :].rearrange("p c x -> p (c x)")
        exh = ex[:, c0:c1, :].rearrange("p c x -> p (c x)")
        nc.vector.tensor_mul(exh, txh, txh)
        expx_i = nc.scalar.activation(exh, exh, AF.Exp, scale=-0.5)
        bass_rust.add_dep_helper(expx_i.ins, expy_i.ins, sync=False,
                                 reason="Exp_y feeds the PE-critical lhsT path")

    # channel weights q[:, c, j]: j=0 -> opac, j=1..3 -> opac*color
    # (emitted late: only needed by wmat)
    qw = sb.tile([128, NCHUNK, 4], f32, tag="qw")
    nc.gpsimd.tensor_copy(qw[:, :, 0], gp[:, :, 1])
    for ch in range(3):
        nc.gpsimd.tensor_mul(qw[:, :, ch + 1], gp[:, :, 1], gp[:, :, ch + 2])

    # ---------------- channel-scaled Ey (GPSIMD) ----------------
    # wmat[:, c, j, :] = qw[:, c, j] * ey[:, c, :]  via broadcast reads
    wmat = sb.tile([128, NCHUNK, 4, YQ], f32r, tag="wmat")

    def wmat_op(eng, c0, c1):
        eyc = ey[:, c0:c1, :]
        ey_b = bass.AP(
            tensor=ey.tensor, offset=eyc.offset,
            ap=[eyc.ap[0], eyc.ap[1], [0, 4], eyc.ap[2]],
        )
        qwc = qw[:, c0:c1, :]
        q_b = bass.AP(
            tensor=qw.tensor, offset=qwc.offset,
            ap=[qwc.ap[0], qwc.ap[1], qwc.ap[2], [0, YQ]],
        )
        eng.tensor_mul(wmat[:, c0:c1, :, :], ey_b, q_b)

    wmat_op(nc.gpsimd, 0, 2)
    wmat_op(nc.gpsimd, 2, 4)
    wmat_op(nc.gpsimd, 4, 6)
    wmat_op(nc.vector, 6, 8)

    # ---------------- render matmuls (PE, float32r) ----------------
    accs = [ps.tile([YQ, W], f32, tag=f"acc{j}", name=f"acc{j}") for j in range(4)]
    for c in range(NCHUNK):
        for j in range(4):
            nc.tensor.matmul(
                accs[j][:, :],
                lhsT=wmat[:, c, j, :],
                rhs=ex[:, c, :],
                start=(c == 0),
                stop=(c == NCHUNK - 1),
            )

    # ---------------- epilogue ----------------
    rden = sb.tile([YQ, W], f32, tag="rden")
    nc.vector.tensor_scalar_add(rden[:, :], accs[0][:, :], EPS)
    nc.vector.reciprocal(rden[:, :], rden[:, :])
    outt = sb.tile([YQ, 3, W], f32, tag="outt")
    for ch in range(3):
        nc.vector.tensor_mul(outt[:, ch, :], accs[ch + 1][:, :], rden[:, :])
    nc.sync.dma_start(
        out=out[0:2, :, :].rearrange("c y x -> y c x"), in_=outt[:, 0:2, :]
    )
    nc.scalar.dma_start(out=out[2, :, :], in_=outt[:, 2, :])


def _build_module():
    nc = bacc.Bacc("TRN2", target_bir_lowering=False, debug=False)
    with tile.TileContext(nc) as tc:
        with (
            tc.tile_pool(name="sb", bufs=1) as sb,
            tc.tile_pool(name="ps", bufs=1, space="PSUM") as ps,
        ):
            build_kernel(nc, sb, ps)
    nc.compile()
    return nc


def _host_pm(qvec_b: np.ndarray, tvec_b: np.ndarray) -> np.ndarray:
    """Combined projection matrix: [x y z 1] @ pm = (u, v, z_cam) with
    proj = (u/z_cam, v/z_cam).  Mirrors reference._quat_to_rot."""
    q = qvec_b.astype(np.float64)
    q = q / np.linalg.norm(q)
    w_, x, y, z = q
    R = np.array(
        [
            [1 - 2 * (y * y + z * z), 2 * (x * y - z * w_), 2 * (x * z + y * w_)],
            [2 * (x * y + z * w_), 1 - 2 * (x * x + z * z), 2 * (y * z - x * w_)],
            [2 * (x * z - y * w_), 2 * (y * z + x * w_), 1 - 2 * (x * x + y * y)],
        ]
    )
    t = tvec_b.astype(np.float64)
    rows = np.stack([FX * R[0] + CX * R[2], FY * R[1] + CY * R[2], R[2]], axis=1)
    last = np.array([FX * t[0] + CX * t[2], FY * t[1] + CY * t[2], t[2]])
    return np.concatenate([rows, last[None, :]], axis=0).astype(np.float32)


def kernel(positions, colors, opacities, scales, qvec, tvec, pixel_coords):
    global _CACHED_NC, LAST_RESULTS
    if _CACHED_NC is None:
        _CACHED_NC = _build_module()
    nc = _CACHED_NC

    f32 = np.float32
    pos = np.asarray(positions, f32)
    colv = np.asarray(colors, f32)
    opv = np.asarray(opacities, f32).reshape(N)
    scv = np.asarray(scales, f32).reshape(N)
    pc = np.asarray(pixel_coords, f32).reshape(H, W, 2)
    xs = np.ascontiguousarray(pc[0, :, 0].reshape(1, W))
    ys_full = pc[:, 0, 1]

    # host-side packing (layout only; all math stays on device)
    posT_h = np.ascontiguousarray(
        np.concatenate([pos.T, np.ones((1, N), f32)], axis=0)
    )  # [4, N]
    gprops_h = np.ascontiguousarray(
        np.concatenate([scv.reshape(N, 1), opv.reshape(N, 1), colv], axis=1)
        .reshape(NCHUNK, 128, 5)
        .transpose(1, 0, 2)
    )  # [128, NCHUNK, 5]

    pms = [
        _host_pm(np.asarray(qvec, f32)[b], np.asarray(tvec, f32)[b]) for b in range(B)
    ]

    in_maps = []
    for core in range(NCORES):
        b, qy = divmod(core, 4)
        aux = np.concatenate(
            [pms[b].reshape(-1), xs.reshape(-1), ys_full[qy * YQ : (qy + 1) * YQ]]
        ).reshape(1, 12 + W + YQ)
        in_maps.append(
            dict(
                post=posT_h,
                gprops=gprops_h,
                aux=np.ascontiguousarray(aux, dtype=f32),
            )
        )

    res = run_bass_kernel_spmd(nc, in_maps, core_ids=list(range(NCORES)), trace=TRACE)
    LAST_RESULTS = res

    outv = np.zeros((B, 3, H, W), f32)
    for core in range(NCORES):
        b, qy = divmod(core, 4)
        outv[b, :, qy * YQ : (qy + 1) * YQ, :] = res.results[core]["out"]
    return outv
